# revision 13
# baseline (speedup 1.0000x reference)
"""VQ codebook forward (cdist^2 + argmin + gather + commitment loss) on 8 TRN2 cores.

Strategy (data-parallel per sharding hint):
  - Shard z along the flattened token axis: 4096 tokens per core; replicate the
    8 MB codebook.
  - Per core: scores s'[n,k] = z.c_k - ||c_k||^2/2 via fp32r matmuls (full PE
    rate) + a K=1 bf16 matmul folding the -||c||^2/2 row into PSUM.
  - DVE max/max_index give the top-8 candidate codes per token under fp32r
    precision; the true argmin is recovered exactly by re-ranking the top-4
    candidates with precise fp32 dot products (q_j = z.c_j - cb2_j/2), which
    reproduces the reference's fp32 comparison structure.
  - z_q rows come from an indirect DMA gather of the codebook; the commitment
    loss uses d_min = ||z||^2 - 2*q_max, summed on host in float64.
"""

import sys

sys.path.insert(0, "/opt/trn_rl_repo")

from contextlib import ExitStack

import numpy as np

import concourse.bacc as bacc
import concourse.bass as bass
import concourse.mybir as mybir
from concourse.bass import IndirectOffsetOnAxis
from concourse.bass_utils import run_bass_kernel_spmd
from concourse.masks import make_identity
from concourse.tile import TileContext

B, S, D = 32, 1024, 256
K = 8192
NCORES = 8
NTOK = B * S
NC_TOK = NTOK // NCORES  # 4096 tokens per core
MT_FULL = NC_TOK // 128  # 32 m-tiles per core
NCHUNK = K // 512  # 16 chunks of 512 codes
COMMIT = 0.25

F32 = mybir.dt.float32
F32R = mybir.dt.float32r
BF16 = mybir.dt.bfloat16
U32 = mybir.dt.uint32
I32 = mybir.dt.int32
AX = mybir.AxisListType
OP = mybir.AluOpType
ACTF = mybir.ActivationFunctionType


def build(mt: int = MT_FULL):
    """Build the per-core Bass module. `mt` = number of 128-token m-tiles."""
    ntok = mt * 128
    nc = bacc.Bacc()

    z_in = nc.declare_dram_parameter("z", [ntok, D], F32, isOutput=False)
    cb_in = nc.declare_dram_parameter("cb", [K, D], F32, isOutput=False)
    zq_out = nc.declare_dram_parameter("zq", [ntok, D], F32, isOutput=True)
    ids_out = nc.declare_dram_parameter("ids", [ntok, 1], I32, isOutput=True)
    dmin_out = nc.declare_dram_parameter("dmin", [ntok, 1], F32, isOutput=True)

    # bounce buffer in (p, kb) layout: element kb*128+p at offset p*64+kb
    cb2_bounce = nc.dram_tensor("cb2_bounce", [128, K // 128], F32)

    with TileContext(nc) as tc, ExitStack() as ctx:
        statics = ctx.enter_context(tc.tile_pool(name="statics", bufs=1))
        spool = ctx.enter_context(tc.tile_pool(name="spool", bufs=2))
        io2 = ctx.enter_context(tc.tile_pool(name="io2", bufs=2))
        small = ctx.enter_context(tc.tile_pool(name="small", bufs=2))
        psum = ctx.enter_context(tc.tile_pool(name="psum", bufs=2, space="PSUM"))

        # ---- static tiles ----
        cbT_r = statics.tile([128, 2, K], F32R)  # codebook^T, fp32r
        zT_r = statics.tile([128, 2, mt, 128], F32R)  # z^T, fp32r
        cb2t = statics.tile([128, 4, 512], BF16)  # -cb2/2 rows at parts {0,32,64,96}
        ones_bf = statics.tile([128, 128], BF16)
        ident = statics.tile([128, 128], F32)
        iota8 = statics.tile([128, 8], U32)
        iota8f = statics.tile([128, 8], F32)
        cb2_pd = statics.tile([128, K // 128], F32)  # (p, kb) -> cb2[kb*128+p]
        q8 = statics.tile([128, 8], F32)  # candidate scores; slots 4-7 = -inf

        nc.vector.memset(q8, -3.0e38)
        nc.vector.memset(ones_bf, 1.0)
        make_identity(nc, ident)
        nc.gpsimd.iota(iota8, pattern=[[1, 8]], base=0, channel_multiplier=0)
        nc.vector.tensor_copy(out=iota8f, in_=iota8)

        # ---- pre-phase: codebook transpose + cb2 ----
        for kb in range(K // 128):
            cbn = io2.tile([128, D], F32, tag="cbn")
            nc.sync.dma_start(out=cbn, in_=cb_in[kb * 128 : (kb + 1) * 128])
            # cb2 partial: sum of squares along D
            junk = io2.tile([128, D], F32, tag="junk")
            nc.scalar.activation(
                out=junk, in_=cbn, func=ACTF.Square,
                accum_out=cb2_pd[:, kb : kb + 1],
            )
            for j in range(2):
                pt = psum.tile([128, 128], F32, tag="mm")
                nc.tensor.transpose(
                    out=pt, in_=cbn[:, j * 128 : (j + 1) * 128], identity=ident
                )
                nc.scalar.copy(out=cbT_r[:, j, kb * 128 : (kb + 1) * 128], in_=pt)

        # scale to -cb2/2 and bounce out in two layouts
        nc.vector.tensor_scalar_mul(cb2_pd, cb2_pd, -0.5)
        # contiguous per-partition bounce (efficient): [128, 64]
        nc.gpsimd.dma_start(out=cb2_bounce[:, :], in_=cb2_pd)
        # read back into the row-tiled bf16 layout:
        # cb2t[32*j4, g, i] = -cb2/2 of code (4g+j4)*512 + i
        # code k = c*512 + i  ->  bounce[(i%128), 4c + i//128]
        bview = cb2_bounce.rearrange("p kb -> (p kb)")
        for j4 in range(4):
            for g in range(4):
                c = 4 * g + j4
                # offsets: base 4c + i_hi*1 (i_hi: 4) + i_lo*64 (i_lo: 128)
                src = bass.AP(
                    tensor=bview.tensor,
                    offset=4 * c,
                    ap=[[0, 1], [1, 4], [64, 128]],
                )
                dst = cb2t[32 * j4 : 32 * j4 + 1, g].rearrange(
                    "p (hi lo) -> p hi lo", hi=4
                )
                nc.gpsimd.dma_start(out=dst, in_=src)

        # ---- pre-phase: z transpose ----
        for m in range(mt):
            zn = io2.tile([128, D], F32, tag="zn")
            nc.sync.dma_start(out=zn, in_=z_in[m * 128 : (m + 1) * 128])
            for j in range(2):
                pt = psum.tile([128, 128], F32, tag="mm")
                nc.tensor.transpose(
                    out=pt, in_=zn[:, j * 128 : (j + 1) * 128], identity=ident
                )
                nc.scalar.copy(out=zT_r[:, j, m, :], in_=pt)

        # ---- steady loop ----
        for m in range(mt):
            sp = spool.tile([128, K], F32, tag="s")
            for g in range(4):
                pg = psum.tile([128, 2048], F32, tag="mm")
                for j4 in range(4):
                    c = 4 * g + j4
                    col = slice(j4 * 512, (j4 + 1) * 512)
                    nc.tensor.matmul(
                        out=pg[:, col],
                        lhsT=ones_bf[32 * j4 : 32 * j4 + 1, :],
                        rhs=cb2t[32 * j4 : 32 * j4 + 1, g],
                        start=True, stop=False,
                        tile_position=(32 * j4, 0),
                    )
                    for j in range(2):
                        nc.tensor.matmul(
                            out=pg[:, col],
                            lhsT=zT_r[:, j, m, :],
                            rhs=cbT_r[:, j, c * 512 : (c + 1) * 512],
                            start=False, stop=(j == 1),
                        )
                nc.scalar.copy(out=sp[:, g * 2048 : (g + 1) * 2048], in_=pg)

            # top-8 candidate codes under fp32r precision
            top8v = small.tile([128, 8], F32, tag="top8v")
            top8i = small.tile([128, 8], U32, tag="top8i")
            nc.vector.max(out=top8v, in_=sp)
            nc.vector.max_index(out=top8i, in_max=top8v, in_values=sp)

            # gather candidate vectors + their -cb2/2 + fresh z
            cand = io2.tile([128, 4, D], F32, tag="cand")
            for j in range(4):
                nc.gpsimd.indirect_dma_start(
                    out=cand[:, j],
                    out_offset=None,
                    in_=cb_in[:],
                    in_offset=IndirectOffsetOnAxis(ap=top8i[:, j : j + 1], axis=0),
                )
            zn = io2.tile([128, D], F32, tag="zn")
            nc.sync.dma_start(out=zn, in_=z_in[m * 128 : (m + 1) * 128])

            # precise per-candidate dot products, candidate norms, ||z||^2
            prods = io2.tile([128, 4, D], F32, tag="prods")
            znb = zn.rearrange("p (o d) -> p o d", o=1).to_broadcast([128, 4, D])
            nc.gpsimd.tensor_tensor(out=prods, in0=znb, in1=cand, op=OP.mult)
            s4 = small.tile([128, 4], F32, tag="s4")
            c2 = small.tile([128, 4], F32, tag="c2")
            junk2 = io2.tile([128, D], F32, tag="junk2")
            for j in range(4):
                nc.scalar.activation(
                    out=junk2, in_=prods[:, j], func=ACTF.Copy,
                    accum_out=s4[:, j : j + 1],
                )
                nc.scalar.activation(
                    out=junk2, in_=cand[:, j], func=ACTF.Square,
                    accum_out=c2[:, j : j + 1],
                )
            z2 = small.tile([128, 1], F32, tag="z2")
            junk3 = io2.tile([128, D], F32, tag="junk3")
            nc.scalar.activation(
                out=junk3, in_=zn, func=ACTF.Square, accum_out=z2,
            )

            # q_j = s_j - c2_j/2 ; argmax over the 4 candidates = exact argmin d
            nc.vector.scalar_tensor_tensor(
                out=q8[:, 0:4], in0=c2, scalar=-0.5, in1=s4,
                op0=OP.mult, op1=OP.add,
            )
            qtop = small.tile([128, 8], F32, tag="qtop")
            jtop = small.tile([128, 8], U32, tag="jtop")
            nc.vector.max(out=qtop, in_=q8)
            nc.vector.max_index(out=jtop, in_max=qtop, in_values=q8)

            # d_min = z2 - 2*q_max  (for the commitment loss)
            dmin = small.tile([128, 1], F32, tag="dmin")
            nc.vector.tensor_scalar(
                out=dmin, in0=qtop[:, 0:1], scalar1=-2.0, scalar2=z2,
                op0=OP.mult, op1=OP.add,
            )
            nc.sync.dma_start(
                out=dmin_out[m * 128 : (m + 1) * 128], in_=dmin
            )

            # select the winning candidate's id: sum_j ids[j] * (j == j*)
            jf = small.tile([128, 1], F32, tag="jf")
            nc.vector.tensor_copy(out=jf, in_=jtop[:, 0:1])
            idsf = small.tile([128, 4], F32, tag="idsf")
            nc.vector.tensor_copy(out=idsf, in_=top8i[:, 0:4])
            idmul = small.tile([128, 4], F32, tag="idmul")
            idf = small.tile([128, 1], F32, tag="idf")
            nc.vector.scalar_tensor_tensor(
                out=idmul, in0=iota8f[:, 0:4], scalar=jf, in1=idsf,
                op0=OP.is_equal, op1=OP.mult, accum_out=idf,
            )
            idi = small.tile([128, 1], I32, tag="idi")
            nc.vector.tensor_copy(out=idi, in_=idf)
            nc.sync.dma_start(out=ids_out[m * 128 : (m + 1) * 128], in_=idi)

            # z_q gather + writeback
            zq = io2.tile([128, D], F32, tag="zq")
            nc.gpsimd.indirect_dma_start(
                out=zq,
                out_offset=None,
                in_=cb_in[:],
                in_offset=IndirectOffsetOnAxis(ap=idi, axis=0),
            )
            nc.sync.dma_start(out=zq_out[m * 128 : (m + 1) * 128], in_=zq)

    nc.finalize()
    return nc


_NC_CACHE = {}


def _get_nc(mt: int = MT_FULL):
    if mt not in _NC_CACHE:
        _NC_CACHE[mt] = build(mt)
    return _NC_CACHE[mt]


def kernel(z: np.ndarray, codebook: np.ndarray, _trace: bool = False):
    z = np.asarray(z, dtype=np.float32)
    cb = np.ascontiguousarray(np.asarray(codebook, dtype=np.float32))
    zf = np.ascontiguousarray(z.reshape(-1, D))

    nc = _get_nc()
    in_maps = [
        {"z": np.ascontiguousarray(zf[c * NC_TOK : (c + 1) * NC_TOK]), "cb": cb}
        for c in range(NCORES)
    ]
    res = run_bass_kernel_spmd(
        nc, in_maps, core_ids=list(range(NCORES)), trace=_trace
    )

    zq_g = np.concatenate([r["zq"] for r in res.results], axis=0)  # (NTOK, D)
    ids_g = np.concatenate([r["ids"][:, 0] for r in res.results], axis=0)
    dmin_g = np.concatenate([r["dmin"][:, 0] for r in res.results], axis=0)

    # straight-through output exactly as the reference computes it:
    # z_q_flat = z + (z_q - z) in fp32
    zq_st = (zf + (zq_g - zf)).reshape(B, S, D)
    ids = ids_g.astype(np.int32).reshape(B, S)
    loss = np.float32(COMMIT * (np.sum(dmin_g, dtype=np.float64) / (NTOK * D)))
    if _trace:
        return (zq_st, ids, loss), res
    return zq_st, ids, loss
